# revision 25
# baseline (speedup 1.0000x reference)
"""Multi-head attention TRN2 kernel (B=4, S=2048, E=1024, H=16, D=64) on 8 cores.

Sharding: core c handles (batch b = c//2, query-half hq = c%2). Each core gets
the full batch-b sequence (rotated so its query half is rows 0-1023 -- softmax
over keys is order-invariant) and computes y rows for its 1024 queries. No
collectives; outputs concatenate.

Per-core dataflow (projection matmuls in float32r -- fp32 data at bf16 PE rate):
  1. Fused: per s-tile, transpose x to xT (e on partitions) and immediately
     V = x @ Wv + bv for the previous s-tile -> DRAM scratch (bf16).
  2. Per head-pair p (heads 2p, 2p+1): QT_p [d2, 1024], KT_p [d2, 2048] from
     lhsT=W slices, rhs=xT (bias added via scalar ACT at PSUM eviction).
     Scores^T [k, q] per k-tile as a row-tiled pair (K=64 at array rows
     0-63 / 64-127, concurrent).  exp split across two engines: scalar ACT
     (exact, exp(s/8 + ln lam)) and a custom DVE op computing
     lam*e^(s/8) ~ ((s-R)(s^2+Bs+G))^4 -- same lam, so softmax is exact up
     to the poly ripple on DVE k-tiles.  attnV with ones-augmented V
     stationary [k, 65] (bf16): rows 0-63 = attn_outT, row 64 = softmax
     denominator, accumulated over 16 k-tiles in PSUM.
  3. Normalize at eviction via the DRAM-roundtrip reciprocal dance
     (off the critical path, on the gpsimd DMA queue).
  4. y = attn_outT.T @ W_out + b_out per 128-row q-tile (W_out half 0
     prefetched during phase B, half 1 loaded under phase C half-0 compute).
"""
from contextlib import ExitStack

import numpy as np

import concourse.bass as bass
import concourse.tile as tile
from concourse import bacc, mybir, dve_ops
from concourse.bass_utils import run_bass_kernel_spmd
from concourse.dve_spec import C0, C1, C2, Spec, Src0, sq
from concourse.masks import make_identity

F32R = mybir.dt.float32r
F32 = mybir.dt.float32
BF16 = mybir.dt.bfloat16
AF = mybir.ActivationFunctionType

B, S, E, H, D = 4, 2048, 1024, 16, 64
Q = 1024          # queries per core
ET = 8            # e-tiles (contraction over E)
ST = 16           # s-tiles of the sequence
KT = 16           # k-tiles in attention
NP = 8            # head-pairs
N_CORES = 8

# lam*e^(s/8) ~ ((s - R)(s^2 + Bc*s + G))^4 for raw scores s in [-20, 20].
EXP_R = -5.36194375e+01
EXP_B = 4.77311991e+01
EXP_G = 3.76928874e+03
LN_LAM = 48.86652111696897   # scalar-ACT bias so both exp paths share lam
# k-tiles whose exp runs on the DVE custom op (rest: scalar ACT, exact)
DVE_KS = frozenset({1, 3, 5, 7, 9, 11, 13})


def _exp4_ref(in0, in1, s0, s1, imm2):
    pp = ((in0 - s0) * ((in0 * in0 + s1 * in0) + imm2)).astype(np.float32)
    y = (pp * pp).astype(np.float32)
    return (y * y).astype(np.float32)


def _register_exp4():
    for op in dve_ops.OPS:
        if op.name == "EXP4_ANT":
            return op
    x = Src0
    body = sq(sq((x - C0) * ((sq(x) + C1 * x) + C2)))
    op = dve_ops.DveOp(
        "EXP4_ANT",
        Spec(body=body, reference=_exp4_ref),
        subdim=False,
        uops_sha={"v3": "8097539a72e1c183"},
    )
    dve_ops.OPS.append(op)
    dve_ops.CUSTOM_DVE_SPECS[op.name] = op.spec
    dve_ops._SUB_OPCODE_FOR_NAME[op.name] = (
        dve_ops._CUSTOM_DVE_ROW_BASE + len(dve_ops.OPS) - 1
    )
    return op


EXP4 = _register_exp4()


def _bcast_dram(ap1d, n_part, n_free):
    """Broadcast a DRAM row across n_part partitions: [[0,n_part],[1,n_free]]."""
    return bass.AP(
        tensor=ap1d.tensor, offset=ap1d.offset, ap=[[0, n_part], [1, n_free]]
    )


def _emit(tc, nc, x, wqkv, bqkv, wout, bout, y, vdram, rscr, rscr2):
    with ExitStack() as ctx:
        xt_pool = ctx.enter_context(tc.tile_pool(name="xt", bufs=1))
        const = ctx.enter_context(tc.tile_pool(name="const", bufs=1))

        xt = xt_pool.tile([128, ST, ET, 128], BF16)

        # ---- fused phase 0+A: x -> xT, V = x @ Wv + bv -> vdram ----
        wv_ctx = tc.tile_pool(name="wv", bufs=1)
        wvp = wv_ctx.__enter__()
        wv = wvp.tile([128, ET, E], F32R)
        nc.scalar.dma_start(
            out=wv, in_=wqkv[:, 2 * E : 3 * E].rearrange("(t p) n -> p t n", p=128)
        )
        bqk_t = const.tile([128, 24], F32)
        nc.gpsimd.dma_start(
            out=bqk_t, in_=bqkv.rearrange("(j p) -> p j", p=128).bitcast(F32)
        )
        lnlam_t = const.tile([128, 1], F32)
        nc.vector.memset(lnlam_t, LN_LAM)

        with (
            tc.tile_pool(name="fconst", bufs=1) as fconst,
            tc.tile_pool(name="xload", bufs=5) as xload,
            tc.tile_pool(name="tps", bufs=2, space="PSUM") as tps,
            tc.tile_pool(name="vps", bufs=3, space="PSUM") as vps,
            tc.tile_pool(name="vev", bufs=4) as vev,
        ):
            ident_f = fconst.tile([128, 128], F32)
            make_identity(nc, ident_f)
            ident = fconst.tile([128, 128], BF16)
            nc.vector.tensor_copy(ident, ident_f)
            wvb = fconst.tile([128, ET, E], BF16)
            nc.vector.tensor_copy(wvb, wv)
            bv_t = fconst.tile([128, E], F32R)
            nc.gpsimd.dma_start(
                out=bv_t, in_=_bcast_dram(bqkv[2 * E : 2 * E + 1], 128, E)
            )

            def v_proj(st):
                ps = vps.tile([128, E], F32)
                for half in range(2):
                    for et in range(ET):
                        nc.tensor.matmul(
                            ps[:, half * 512 : (half + 1) * 512],
                            xt[:, st, et, :],
                            wvb[:, et, half * 512 : (half + 1) * 512],
                            start=(et == 0),
                            stop=(et == ET - 1),
                        )
                vb = vev.tile([128, E], BF16)
                nc.vector.tensor_add(vb, ps, bv_t)
                nc.gpsimd.dma_start(out=vdram[st * 128 : (st + 1) * 128, :], in_=vb)

            for st in range(ST):
                xsf = xload.tile([128, E], F32R, tag="xsf")
                nc.sync.dma_start(out=xsf, in_=x[st * 128 : (st + 1) * 128, :])
                xs = xload.tile([128, E], BF16, tag="xs")
                nc.vector.tensor_copy(xs, xsf)
                for g in range(2):
                    ps = tps.tile([128, 4, 128], BF16)
                    for i in range(4):
                        et = g * 4 + i
                        nc.tensor.transpose(
                            ps[:, i, :], xs[:, et * 128 : (et + 1) * 128], ident
                        )
                    nc.vector.tensor_copy(xt[:, st, g * 4 : (g + 1) * 4, :], ps)
                if st >= 1:
                    v_proj(st - 1)
            v_proj(ST - 1)
        wv_ctx.__exit__(None, None, None)

        # ---- phase B: per-pair QK JIT + attention, software-pipelined ----
        aout_pool = ctx.enter_context(tc.tile_pool(name="aout", bufs=1))
        aout = aout_pool.tile([128, NP, Q], BF16)
        wo_pool = ctx.enter_context(tc.tile_pool(name="wo", bufs=1))
        wo = {}

        def load_wo(half):
            wof = wo_pool.tile([128, ET, 512], F32R, name=f"wof{half}", tag="wof")
            wo[half] = wo_pool.tile([128, ET, 512], BF16, name=f"wo{half}")
            for p8 in range(8):
                nc.scalar.dma_start(
                    out=wof[:, p8, :],
                    in_=wout[
                        p8 * 128 : (p8 + 1) * 128, half * 512 : (half + 1) * 512
                    ],
                )
            nc.vector.tensor_copy(wo[half], wof)

        with (
            tc.tile_pool(name="wqk", bufs=2) as wqkp,
            tc.tile_pool(name="qt", bufs=2) as qtp,
            tc.tile_pool(name="kt", bufs=2) as ktp,
            tc.tile_pool(name="vp", bufs=2) as vpp,
            tc.tile_pool(name="pt", bufs=4) as ptp,
            tc.tile_pool(name="ev", bufs=2) as evp,
            tc.tile_pool(name="qkps", bufs=1, space="PSUM") as qkps,
            tc.tile_pool(name="scps", bufs=2, space="PSUM") as scps,
            tc.tile_pool(name="accps", bufs=2, space="PSUM") as accps,
        ):

            def build_pair(p):
                """Allocate pair-p input tiles; return (tiles, emission thunks)."""
                wqf = wqkp.tile([128, ET, 128], F32R, tag="wqf")
                wkf = wqkp.tile([128, ET, 128], F32R, tag="wkf")
                wq = wqkp.tile([128, ET, 128], BF16, tag="wq")
                wk = wqkp.tile([128, ET, 128], BF16, tag="wk")
                qt_t = qtp.tile([128, Q], BF16)
                kt_t = ktp.tile([128, S], BF16)
                vp = vpp.tile([128, KT, 2, 65], BF16)
                th = []
                th.append(lambda: nc.scalar.dma_start(
                    out=wqf,
                    in_=wqkv[:, p * 128 : (p + 1) * 128].rearrange(
                        "(t p2) m -> p2 t m", p2=128),
                ))
                th.append(lambda: nc.vector.tensor_copy(wq, wqf))
                th.append(lambda: nc.scalar.dma_start(
                    out=wkf,
                    in_=wqkv[:, E + p * 128 : E + (p + 1) * 128].rearrange(
                        "(t p2) m -> p2 t m", p2=128),
                ))
                th.append(lambda: nc.vector.tensor_copy(wk, wkf))
                for h in range(2):
                    th.append(lambda h=h: nc.sync.dma_start(
                        out=vp[:, :, h, 0:64],
                        in_=vdram[
                            :, p * 128 + h * 64 : p * 128 + h * 64 + 64
                        ].rearrange("(t p2) d -> p2 t d", p2=128),
                    ))
                th.append(lambda: nc.vector.memset(vp[:, :, :, 64:65], 1.0))

                def qk_group(dst, w, bias_col, xoff):
                    g = []
                    ps_box = []

                    def alloc():
                        qk_ps = qkps.tile([128, 1024], F32, name="qk_ps", tag="qk")
                        ps_box.append(qk_ps)
                    g.append(alloc)
                    for half in range(2):
                        for et in range(ET):
                            g.append(lambda half=half, et=et: nc.tensor.matmul(
                                ps_box[0][:, half * 512 : (half + 1) * 512],
                                w[:, et, :],
                                xt[:, (xoff + half * 512) // 128 : (xoff + half * 512) // 128 + 4, et, :],
                                start=(et == 0),
                                stop=(et == ET - 1),
                            ))
                    g.append(lambda: nc.scalar.activation(
                        out=dst, in_=ps_box[0], func=AF.Identity,
                        bias=bias_col, scale=1.0,
                    ))
                    return g

                th += qk_group(qt_t[:, :], wq, bqk_t[:, p : p + 1], 0)
                th += qk_group(kt_t[:, 0:1024], wk, bqk_t[:, 8 + p : 9 + p], 0)
                th += qk_group(kt_t[:, 1024:2048], wk, bqk_t[:, 8 + p : 9 + p], 1024)
                return {"qt": qt_t, "kt": kt_t, "vp": vp}, th

            cur, th0 = build_pair(0)
            for t in th0:
                t()

            for p in range(NP):
                if p + 1 < NP:
                    nxt, pending = build_pair(p + 1)
                else:
                    nxt = None
                    pending = []
                pending = list(pending)
                if p == 2:
                    load_wo(0)
                elif p == 6:
                    load_wo(1)
                qt_t, kt_t, vp = cur["qt"], cur["kt"], cur["vp"]
                for qh in range(2):
                    qsl = slice(qh * 512, (qh + 1) * 512)
                    acc0 = accps.tile([128, 512], F32, tag="acc")
                    acc1 = accps.tile([128, 512], F32, tag="acc")
                    pts = [None] * KT
                    for k in range(KT):
                        sc = scps.tile([128, 1024], F32, tag="sc")
                        nc.tensor.matmul(
                            sc[:, 0:512],
                            kt_t[0:64, k * 128 : (k + 1) * 128],
                            qt_t[0:64, qsl],
                            start=True, stop=True,
                        )
                        nc.tensor.matmul(
                            sc[:, 512:1024],
                            kt_t[64:128, k * 128 : (k + 1) * 128],
                            qt_t[64:128, qsl],
                            start=True, stop=True,
                        )
                        if k >= 1:
                            pt_p = pts[k - 1]
                            nc.tensor.matmul(
                                acc0[0:65, :], vp[:, k - 1, 0, :], pt_p[:, 0:512],
                                start=(k - 1 == 0), stop=(k - 1 == KT - 1),
                            )
                            nc.tensor.matmul(
                                acc1[0:65, :], vp[:, k - 1, 1, :], pt_p[:, 512:1024],
                                start=(k - 1 == 0), stop=(k - 1 == KT - 1),
                            )
                        pt_t = ptp.tile([128, 1024], BF16)
                        pts[k] = pt_t
                        if k in DVE_KS:
                            nc.vector._custom_dve(
                                EXP4, out=pt_t[:, :], in0=sc,
                                s0=EXP_R, s1=EXP_B, imm2=EXP_G,
                            )
                        else:
                            nc.scalar.activation(
                                out=pt_t, in_=sc, func=AF.Exp,
                                scale=0.125, bias=lnlam_t[:, 0:1],
                            )
                        for _ in range(2):
                            if pending:
                                pending.pop(0)()
                    nc.tensor.matmul(
                        acc0[0:65, :], vp[:, KT - 1, 0, :], pts[KT - 1][:, 0:512],
                        start=False, stop=True,
                    )
                    nc.tensor.matmul(
                        acc1[0:65, :], vp[:, KT - 1, 1, :], pts[KT - 1][:, 512:1024],
                        start=False, stop=True,
                    )
                    # eviction: fast psum release, then off-path normalization
                    ridx = p * 2 + qh
                    au0 = evp.tile([128, 512], F32, tag="au0")
                    nc.vector.tensor_copy(au0[0:65, :], acc0[0:65, :])
                    au1 = evp.tile([128, 512], F32, tag="au1")
                    nc.vector.tensor_copy(au1[0:65, :], acc1[0:65, :])
                    nc.gpsimd.dma_start(out=rscr[ridx : ridx + 1, 0:512], in_=au0[64:65, :])
                    nc.gpsimd.dma_start(out=rscr[ridx : ridx + 1, 512:1024], in_=au1[64:65, :])
                    rw = evp.tile([64, 16], F32, tag="rw")
                    nc.gpsimd.dma_start(
                        out=rw, in_=rscr[ridx : ridx + 1, :].rearrange("o (p f) -> (o p) f", p=64)
                    )
                    rwr = evp.tile([64, 16], F32, tag="rwr")
                    nc.vector.reciprocal(rwr, rw)
                    nc.gpsimd.dma_start(
                        out=rscr2[ridx : ridx + 1, :].rearrange("o (p f) -> (o p) f", p=64),
                        in_=rwr,
                    )
                    sc0 = evp.tile([64, 512], F32, tag="sc0")
                    nc.gpsimd.dma_start(out=sc0, in_=_bcast_dram(rscr2[ridx, 0:1], 64, 512))
                    sc1 = evp.tile([64, 512], F32, tag="sc1")
                    nc.gpsimd.dma_start(out=sc1, in_=_bcast_dram(rscr2[ridx, 512:513], 64, 512))
                    nc.vector.tensor_mul(aout[0:64, p, qsl], au0[0:64, :], sc0)
                    tmp1 = evp.tile([64, 512], BF16, tag="tmp1")
                    nc.vector.tensor_mul(tmp1, au1[0:64, :], sc1)
                    nc.gpsimd.dma_start(out=aout[64:128, p, qsl], in_=tmp1)
                for t in pending:
                    t()
                cur = nxt

        # ---- phase C: y = attn_out @ W_out + b_out ----
        with (
            tc.tile_pool(name="yps", bufs=4, space="PSUM") as yps,
            tc.tile_pool(name="yev", bufs=3) as yev,
        ):
            bout_t = yev.tile([128, E], F32, tag="bout")
            nc.gpsimd.dma_start(out=bout_t, in_=_bcast_dram(bout[0:1], 128, E))
            for half in range(2):
                for qt_i in range(8):
                    ps = yps.tile([128, 512], F32)
                    for p8 in range(8):
                        nc.tensor.matmul(
                            ps,
                            aout[:, p8, qt_i * 128 : (qt_i + 1) * 128],
                            wo[half][:, p8, :],
                            start=(p8 == 0),
                            stop=(p8 == 7),
                        )
                    yb = yev.tile([128, 512], F32)
                    nc.vector.tensor_add(
                        yb, ps, bout_t[:, half * 512 : (half + 1) * 512]
                    )
                    nc.sync.dma_start(
                        out=y[qt_i * 128 : (qt_i + 1) * 128, half * 512 : (half + 1) * 512],
                        in_=yb,
                    )


def build_nc():
    nc = bacc.Bacc("TRN2", target_bir_lowering=False, debug=False)
    x = nc.dram_tensor("x", [S, E], F32R, kind="ExternalInput").ap()
    wqkv = nc.dram_tensor("wqkv", [E, 3 * E], F32R, kind="ExternalInput").ap()
    bqkv = nc.dram_tensor("bqkv", [3 * E], F32R, kind="ExternalInput").ap()
    wout = nc.dram_tensor("wout", [E, E], F32R, kind="ExternalInput").ap()
    bout = nc.dram_tensor("bout", [E], F32, kind="ExternalInput").ap()
    y = nc.dram_tensor("y", [Q, E], F32, kind="ExternalOutput").ap()
    vdram = nc.dram_tensor("vdram", [S, E], BF16).ap()
    rscr = nc.dram_tensor("rscr", [16, 1024], F32).ap()
    rscr2 = nc.dram_tensor("rscr2", [16, 1024], F32).ap()
    with tile.TileContext(nc) as tc:
        _emit(tc, nc, x, wqkv, bqkv, wout, bout, y, vdram, rscr, rscr2)
    nc.compile()
    return nc


_NC = None


def _get_nc():
    global _NC
    if _NC is None:
        _NC = build_nc()
    return _NC


def make_in_maps(x, W_qkv, b_qkv, W_out, b_out):
    x = np.ascontiguousarray(np.asarray(x, dtype=np.float32))
    W_qkv = np.ascontiguousarray(np.asarray(W_qkv, dtype=np.float32))
    b_qkv = np.ascontiguousarray(np.asarray(b_qkv, dtype=np.float32))
    W_out = np.ascontiguousarray(np.asarray(W_out, dtype=np.float32))
    b_out = np.ascontiguousarray(np.asarray(b_out, dtype=np.float32))
    in_maps = []
    for c in range(N_CORES):
        b, hq = c // 2, c % 2
        xb = x[b]
        if hq:
            xb = np.ascontiguousarray(np.concatenate([xb[1024:], xb[:1024]], axis=0))
        in_maps.append(
            {"x": xb, "wqkv": W_qkv, "bqkv": b_qkv, "wout": W_out, "bout": b_out}
        )
    return in_maps


def assemble(results):
    out = np.empty((B, S, E), dtype=np.float32)
    for c in range(N_CORES):
        b, hq = c // 2, c % 2
        out[b, hq * 1024 : (hq + 1) * 1024, :] = results[c]["y"]
    return out


def kernel(x, W_qkv, b_qkv, W_out, b_out):
    nc = _get_nc()
    in_maps = make_in_maps(x, W_qkv, b_qkv, W_out, b_out)
    res = run_bass_kernel_spmd(nc, in_maps, list(range(N_CORES)))
    return assemble(res.results)


# revision 26
# speedup vs baseline: 1.0380x; 1.0380x over previous
"""Multi-head attention TRN2 kernel (B=4, S=2048, E=1024, H=16, D=64) on 8 cores.

Sharding: core c handles (batch b = c//2, query-half hq = c%2). Each core gets
the full batch-b sequence (rotated so its query half is rows 0-1023 -- softmax
over keys is order-invariant) and computes y rows for its 1024 queries. No
collectives; outputs concatenate.

Per-core dataflow (projection matmuls in float32r -- fp32 data at bf16 PE rate):
  1. Fused: per s-tile, transpose x to xT (e on partitions) and immediately
     V = x @ Wv + bv for the previous s-tile -> DRAM scratch (bf16).
  2. Per head-pair p (heads 2p, 2p+1): QT_p [d2, 1024], KT_p [d2, 2048] from
     lhsT=W slices, rhs=xT (bias added via scalar ACT at PSUM eviction).
     Scores^T [k, q] per k-tile as a row-tiled pair (K=64 at array rows
     0-63 / 64-127, concurrent).  exp split across two engines: scalar ACT
     (exact, exp(s/8 + ln lam)) and a custom DVE op computing
     lam*e^(s/8) ~ ((s-R)(s^2+Bs+G))^4 -- same lam, so softmax is exact up
     to the poly ripple on DVE k-tiles.  attnV with ones-augmented V
     stationary [k, 65] (bf16): rows 0-63 = attn_outT, row 64 = softmax
     denominator, accumulated over 16 k-tiles in PSUM.
  3. Normalize at eviction via the DRAM-roundtrip reciprocal dance
     (off the critical path, on the gpsimd DMA queue).
  4. y = attn_outT.T @ W_out + b_out per 128-row q-tile (W_out half 0
     prefetched during phase B, half 1 loaded under phase C half-0 compute).
"""
from contextlib import ExitStack

import numpy as np

import concourse.bass as bass
import concourse.tile as tile
from concourse import bacc, mybir, dve_ops
from concourse.bass_utils import run_bass_kernel_spmd
from concourse.dve_spec import C0, C1, C2, Spec, Src0, sq
from concourse.masks import make_identity

F32R = mybir.dt.float32r
F32 = mybir.dt.float32
BF16 = mybir.dt.bfloat16
AF = mybir.ActivationFunctionType

B, S, E, H, D = 4, 2048, 1024, 16, 64
Q = 1024          # queries per core
ET = 8            # e-tiles (contraction over E)
ST = 16           # s-tiles of the sequence
KT = 16           # k-tiles in attention
NP = 8            # head-pairs
N_CORES = 8

# lam*e^(s/8) ~ ((s - R)(s^2 + Bc*s + G))^4 for raw scores s in [-20, 20].
EXP_R = -5.36194375e+01
EXP_B = 4.77311991e+01
EXP_G = 3.76928874e+03
LN_LAM = 48.86652111696897   # scalar-ACT bias so both exp paths share lam
# k-tiles whose exp runs on the DVE custom op (rest: scalar ACT, exact)
DVE_KS = frozenset({1, 3, 5, 7, 9, 11, 13})


def _exp4_ref(in0, in1, s0, s1, imm2):
    pp = ((in0 - s0) * ((in0 * in0 + s1 * in0) + imm2)).astype(np.float32)
    y = (pp * pp).astype(np.float32)
    return (y * y).astype(np.float32)


def _register_exp4():
    for op in dve_ops.OPS:
        if op.name == "EXP4_ANT":
            return op
    x = Src0
    body = sq(sq((x - C0) * ((sq(x) + C1 * x) + C2)))
    op = dve_ops.DveOp(
        "EXP4_ANT",
        Spec(body=body, reference=_exp4_ref),
        subdim=False,
        uops_sha={"v3": "8097539a72e1c183"},
    )
    dve_ops.OPS.append(op)
    dve_ops.CUSTOM_DVE_SPECS[op.name] = op.spec
    dve_ops._SUB_OPCODE_FOR_NAME[op.name] = (
        dve_ops._CUSTOM_DVE_ROW_BASE + len(dve_ops.OPS) - 1
    )
    return op


EXP4 = _register_exp4()


def _bcast_dram(ap1d, n_part, n_free):
    """Broadcast a DRAM row across n_part partitions: [[0,n_part],[1,n_free]]."""
    return bass.AP(
        tensor=ap1d.tensor, offset=ap1d.offset, ap=[[0, n_part], [1, n_free]]
    )


def _emit(tc, nc, x, wqkv, bqkv, wout, bout, y, vdram, rscr, rscr2):
    with ExitStack() as ctx:
        xt_pool = ctx.enter_context(tc.tile_pool(name="xt", bufs=1))
        const = ctx.enter_context(tc.tile_pool(name="const", bufs=1))

        xt = xt_pool.tile([128, ST, ET, 128], BF16)

        # ---- fused phase 0+A: x -> xT, V = x @ Wv + bv -> vdram ----
        wv_ctx = tc.tile_pool(name="wv", bufs=1)
        wvp = wv_ctx.__enter__()
        wv = wvp.tile([128, ET, E], F32R)
        nc.scalar.dma_start(
            out=wv, in_=wqkv[:, 2 * E : 3 * E].rearrange("(t p) n -> p t n", p=128)
        )
        bqk_t = const.tile([128, 24], F32)
        nc.gpsimd.dma_start(
            out=bqk_t, in_=bqkv.rearrange("(j p) -> p j", p=128).bitcast(F32)
        )
        lnlam_t = const.tile([128, 1], F32)
        nc.vector.memset(lnlam_t, LN_LAM)

        with (
            tc.tile_pool(name="fconst", bufs=1) as fconst,
            tc.tile_pool(name="xload", bufs=5) as xload,
            tc.tile_pool(name="tps", bufs=2, space="PSUM") as tps,
            tc.tile_pool(name="vps", bufs=3, space="PSUM") as vps,
            tc.tile_pool(name="vev", bufs=4) as vev,
        ):
            ident_f = fconst.tile([128, 128], F32)
            make_identity(nc, ident_f)
            ident = fconst.tile([128, 128], BF16)
            nc.vector.tensor_copy(ident, ident_f)
            wvb = fconst.tile([128, ET, E], BF16)
            nc.vector.tensor_copy(wvb, wv)
            bv_t = fconst.tile([128, E], F32R)
            nc.gpsimd.dma_start(
                out=bv_t, in_=_bcast_dram(bqkv[2 * E : 2 * E + 1], 128, E)
            )

            def v_proj(st):
                ps = vps.tile([128, E], F32)
                for half in range(2):
                    for et in range(ET):
                        nc.tensor.matmul(
                            ps[:, half * 512 : (half + 1) * 512],
                            xt[:, st, et, :],
                            wvb[:, et, half * 512 : (half + 1) * 512],
                            start=(et == 0),
                            stop=(et == ET - 1),
                        )
                vb = vev.tile([128, E], BF16)
                nc.vector.tensor_add(vb, ps, bv_t)
                nc.gpsimd.dma_start(out=vdram[st * 128 : (st + 1) * 128, :], in_=vb)

            for st in range(ST):
                xsf = xload.tile([128, E], F32R, tag="xsf")
                nc.sync.dma_start(out=xsf, in_=x[st * 128 : (st + 1) * 128, :])
                xs = xload.tile([128, E], BF16, tag="xs")
                nc.vector.tensor_copy(xs, xsf)
                for g in range(2):
                    ps = tps.tile([128, 4, 128], BF16)
                    for i in range(4):
                        et = g * 4 + i
                        nc.tensor.transpose(
                            ps[:, i, :], xs[:, et * 128 : (et + 1) * 128], ident
                        )
                    nc.vector.tensor_copy(xt[:, st, g * 4 : (g + 1) * 4, :], ps)
                if st >= 1:
                    v_proj(st - 1)
            v_proj(ST - 1)
        wv_ctx.__exit__(None, None, None)

        # ---- phase B: per-pair QK JIT + attention, software-pipelined ----
        aout_pool = ctx.enter_context(tc.tile_pool(name="aout", bufs=1))
        aout = aout_pool.tile([128, NP, Q], BF16)
        wo_pool = ctx.enter_context(tc.tile_pool(name="wo", bufs=1))
        wo = {}

        def load_wo(half):
            wof = wo_pool.tile([128, ET, 512], F32R, name=f"wof{half}", tag="wof")
            wo[half] = wo_pool.tile([128, ET, 512], BF16, name=f"wo{half}")
            for p8 in range(8):
                nc.scalar.dma_start(
                    out=wof[:, p8, :],
                    in_=wout[
                        p8 * 128 : (p8 + 1) * 128, half * 512 : (half + 1) * 512
                    ],
                )
            nc.vector.tensor_copy(wo[half], wof)

        with (
            tc.tile_pool(name="wqk", bufs=2) as wqkp,
            tc.tile_pool(name="qt", bufs=2) as qtp,
            tc.tile_pool(name="kt", bufs=2) as ktp,
            tc.tile_pool(name="vp", bufs=2) as vpp,
            tc.tile_pool(name="pt", bufs=4) as ptp,
            tc.tile_pool(name="ev", bufs=2) as evp,
            tc.tile_pool(name="qkps", bufs=1, space="PSUM") as qkps,
            tc.tile_pool(name="scps", bufs=2, space="PSUM") as scps,
            tc.tile_pool(name="accps", bufs=2, space="PSUM") as accps,
        ):

            def build_pair(p):
                """Allocate pair-p input tiles; return (tiles, emission thunks)."""
                wqf = wqkp.tile([128, ET, 128], F32R, tag="wqf")
                wkf = wqkp.tile([128, ET, 128], F32R, tag="wkf")
                wq = wqkp.tile([128, ET, 128], BF16, tag="wq")
                wk = wqkp.tile([128, ET, 128], BF16, tag="wk")
                qt_t = qtp.tile([128, Q], BF16)
                kt_t = ktp.tile([128, S], BF16)
                vp = vpp.tile([128, KT, 2, 65], BF16)
                th = []
                th.append(lambda: nc.scalar.dma_start(
                    out=wqf,
                    in_=wqkv[:, p * 128 : (p + 1) * 128].rearrange(
                        "(t p2) m -> p2 t m", p2=128),
                ))
                th.append(lambda: nc.vector.tensor_copy(wq, wqf))
                th.append(lambda: nc.scalar.dma_start(
                    out=wkf,
                    in_=wqkv[:, E + p * 128 : E + (p + 1) * 128].rearrange(
                        "(t p2) m -> p2 t m", p2=128),
                ))
                th.append(lambda: nc.vector.tensor_copy(wk, wkf))
                for h in range(2):
                    th.append(lambda h=h: nc.sync.dma_start(
                        out=vp[:, :, h, 0:64],
                        in_=vdram[
                            :, p * 128 + h * 64 : p * 128 + h * 64 + 64
                        ].rearrange("(t p2) d -> p2 t d", p2=128),
                    ))
                th.append(lambda: nc.vector.memset(vp[:, :, :, 64:65], 1.0))

                def qk_group(dst, w, bias_col, xoff):
                    g = []
                    ps_box = []

                    def alloc():
                        qk_ps = qkps.tile([128, 1024], F32, name="qk_ps", tag="qk")
                        ps_box.append(qk_ps)
                    g.append(alloc)
                    for half in range(2):
                        for et in range(ET):
                            g.append(lambda half=half, et=et: nc.tensor.matmul(
                                ps_box[0][:, half * 512 : (half + 1) * 512],
                                w[:, et, :],
                                xt[:, (xoff + half * 512) // 128 : (xoff + half * 512) // 128 + 4, et, :],
                                start=(et == 0),
                                stop=(et == ET - 1),
                            ))
                    g.append(lambda: nc.scalar.activation(
                        out=dst, in_=ps_box[0], func=AF.Identity,
                        bias=bias_col, scale=1.0,
                    ))
                    return g

                th += qk_group(qt_t[:, :], wq, bqk_t[:, p : p + 1], 0)
                th += qk_group(kt_t[:, 0:1024], wk, bqk_t[:, 8 + p : 9 + p], 0)
                th += qk_group(kt_t[:, 1024:2048], wk, bqk_t[:, 8 + p : 9 + p], 1024)
                return {"qt": qt_t, "kt": kt_t, "vp": vp}, th

            cur, th0 = build_pair(0)
            for t in th0:
                t()
            ev_pending = []

            for p in range(NP):
                if p + 1 < NP:
                    nxt, pending = build_pair(p + 1)
                else:
                    nxt = None
                    pending = []
                pending = list(pending)
                if p == 2:
                    load_wo(0)
                elif p == 6:
                    load_wo(1)
                qt_t, kt_t, vp = cur["qt"], cur["kt"], cur["vp"]
                for qh in range(2):
                    qsl = slice(qh * 512, (qh + 1) * 512)
                    acc0 = accps.tile([128, 512], F32, tag="acc")
                    acc1 = accps.tile([128, 512], F32, tag="acc")
                    pts = [None] * KT
                    for k in range(KT):
                        sc = scps.tile([128, 1024], F32, tag="sc")
                        nc.tensor.matmul(
                            sc[:, 0:512],
                            kt_t[0:64, k * 128 : (k + 1) * 128],
                            qt_t[0:64, qsl],
                            start=True, stop=True,
                        )
                        nc.tensor.matmul(
                            sc[:, 512:1024],
                            kt_t[64:128, k * 128 : (k + 1) * 128],
                            qt_t[64:128, qsl],
                            start=True, stop=True,
                        )
                        if k >= 1:
                            pt_p = pts[k - 1]
                            nc.tensor.matmul(
                                acc0[0:65, :], vp[:, k - 1, 0, :], pt_p[:, 0:512],
                                start=(k - 1 == 0), stop=(k - 1 == KT - 1),
                            )
                            nc.tensor.matmul(
                                acc1[0:65, :], vp[:, k - 1, 1, :], pt_p[:, 512:1024],
                                start=(k - 1 == 0), stop=(k - 1 == KT - 1),
                            )
                        pt_t = ptp.tile([128, 1024], BF16)
                        pts[k] = pt_t
                        if k in DVE_KS:
                            nc.vector._custom_dve(
                                EXP4, out=pt_t[:, :], in0=sc,
                                s0=EXP_R, s1=EXP_B, imm2=EXP_G,
                            )
                        else:
                            nc.scalar.activation(
                                out=pt_t, in_=sc, func=AF.Exp,
                                scale=0.125, bias=lnlam_t[:, 0:1],
                            )
                        for _ in range(2):
                            if pending:
                                pending.pop(0)()
                        if k in (4, 6) and ev_pending:
                            ev_pending.pop(0)()
                    nc.tensor.matmul(
                        acc0[0:65, :], vp[:, KT - 1, 0, :], pts[KT - 1][:, 0:512],
                        start=False, stop=True,
                    )
                    nc.tensor.matmul(
                        acc1[0:65, :], vp[:, KT - 1, 1, :], pts[KT - 1][:, 512:1024],
                        start=False, stop=True,
                    )
                    # eviction: fast psum release on scalar; the DVE pieces
                    # of the reciprocal dance are deferred into the next
                    # unit's loop so they never head-of-line block the exps.
                    ridx = p * 2 + qh
                    au0 = evp.tile([128, 512], F32, tag="au0")
                    nc.scalar.copy(au0[0:65, :], acc0[0:65, :])
                    au1 = evp.tile([128, 512], F32, tag="au1")
                    nc.scalar.copy(au1[0:65, :], acc1[0:65, :])
                    nc.gpsimd.dma_start(out=rscr[ridx : ridx + 1, 0:512], in_=au0[64:65, :])
                    nc.gpsimd.dma_start(out=rscr[ridx : ridx + 1, 512:1024], in_=au1[64:65, :])
                    rw = evp.tile([64, 16], F32, tag="rw")
                    nc.gpsimd.dma_start(
                        out=rw, in_=rscr[ridx : ridx + 1, :].rearrange("o (p f) -> (o p) f", p=64)
                    )
                    rwr = evp.tile([64, 16], F32, tag="rwr")

                    def dance1(rw=rw, rwr=rwr, ridx=ridx):
                        nc.vector.reciprocal(rwr, rw)
                        nc.gpsimd.dma_start(
                            out=rscr2[ridx : ridx + 1, :].rearrange("o (p f) -> (o p) f", p=64),
                            in_=rwr,
                        )

                    def dance2(au0=au0, au1=au1, ridx=ridx, p=p, qsl=qsl):
                        sc0 = evp.tile([64, 512], F32, name="sc0", tag="sc0")
                        nc.gpsimd.dma_start(out=sc0, in_=_bcast_dram(rscr2[ridx, 0:1], 64, 512))
                        sc1 = evp.tile([64, 512], F32, name="sc1", tag="sc1")
                        nc.gpsimd.dma_start(out=sc1, in_=_bcast_dram(rscr2[ridx, 512:513], 64, 512))
                        nc.vector.tensor_mul(aout[0:64, p, qsl], au0[0:64, :], sc0)
                        tmp1 = evp.tile([64, 512], BF16, name="tmp1", tag="tmp1")
                        nc.vector.tensor_mul(tmp1, au1[0:64, :], sc1)
                        nc.gpsimd.dma_start(out=aout[64:128, p, qsl], in_=tmp1)

                    ev_pending.append(dance1)
                    ev_pending.append(dance2)
                for t in pending:
                    t()
                cur = nxt
            for t in ev_pending:
                t()
            ev_pending.clear()

        # ---- phase C: y = attn_out @ W_out + b_out ----
        with (
            tc.tile_pool(name="yps", bufs=4, space="PSUM") as yps,
            tc.tile_pool(name="yev", bufs=3) as yev,
        ):
            bout_t = yev.tile([128, E], F32, tag="bout")
            nc.gpsimd.dma_start(out=bout_t, in_=_bcast_dram(bout[0:1], 128, E))
            for half in range(2):
                for qt_i in range(8):
                    ps = yps.tile([128, 512], F32)
                    for p8 in range(8):
                        nc.tensor.matmul(
                            ps,
                            aout[:, p8, qt_i * 128 : (qt_i + 1) * 128],
                            wo[half][:, p8, :],
                            start=(p8 == 0),
                            stop=(p8 == 7),
                        )
                    yb = yev.tile([128, 512], F32)
                    nc.vector.tensor_add(
                        yb, ps, bout_t[:, half * 512 : (half + 1) * 512]
                    )
                    nc.sync.dma_start(
                        out=y[qt_i * 128 : (qt_i + 1) * 128, half * 512 : (half + 1) * 512],
                        in_=yb,
                    )


def build_nc():
    nc = bacc.Bacc("TRN2", target_bir_lowering=False, debug=False)
    x = nc.dram_tensor("x", [S, E], F32R, kind="ExternalInput").ap()
    wqkv = nc.dram_tensor("wqkv", [E, 3 * E], F32R, kind="ExternalInput").ap()
    bqkv = nc.dram_tensor("bqkv", [3 * E], F32R, kind="ExternalInput").ap()
    wout = nc.dram_tensor("wout", [E, E], F32R, kind="ExternalInput").ap()
    bout = nc.dram_tensor("bout", [E], F32, kind="ExternalInput").ap()
    y = nc.dram_tensor("y", [Q, E], F32, kind="ExternalOutput").ap()
    vdram = nc.dram_tensor("vdram", [S, E], BF16).ap()
    rscr = nc.dram_tensor("rscr", [16, 1024], F32).ap()
    rscr2 = nc.dram_tensor("rscr2", [16, 1024], F32).ap()
    with tile.TileContext(nc) as tc:
        _emit(tc, nc, x, wqkv, bqkv, wout, bout, y, vdram, rscr, rscr2)
    nc.compile()
    return nc


_NC = None


def _get_nc():
    global _NC
    if _NC is None:
        _NC = build_nc()
    return _NC


def make_in_maps(x, W_qkv, b_qkv, W_out, b_out):
    x = np.ascontiguousarray(np.asarray(x, dtype=np.float32))
    W_qkv = np.ascontiguousarray(np.asarray(W_qkv, dtype=np.float32))
    b_qkv = np.ascontiguousarray(np.asarray(b_qkv, dtype=np.float32))
    W_out = np.ascontiguousarray(np.asarray(W_out, dtype=np.float32))
    b_out = np.ascontiguousarray(np.asarray(b_out, dtype=np.float32))
    in_maps = []
    for c in range(N_CORES):
        b, hq = c // 2, c % 2
        xb = x[b]
        if hq:
            xb = np.ascontiguousarray(np.concatenate([xb[1024:], xb[:1024]], axis=0))
        in_maps.append(
            {"x": xb, "wqkv": W_qkv, "bqkv": b_qkv, "wout": W_out, "bout": b_out}
        )
    return in_maps


def assemble(results):
    out = np.empty((B, S, E), dtype=np.float32)
    for c in range(N_CORES):
        b, hq = c // 2, c % 2
        out[b, hq * 1024 : (hq + 1) * 1024, :] = results[c]["y"]
    return out


def kernel(x, W_qkv, b_qkv, W_out, b_out):
    nc = _get_nc()
    in_maps = make_in_maps(x, W_qkv, b_qkv, W_out, b_out)
    res = run_bass_kernel_spmd(nc, in_maps, list(range(N_CORES)))
    return assemble(res.results)
